# revision 9
# baseline (speedup 1.0000x reference)
"""Trainium2 Bass kernel for nn_EquivariantWSSHead (gauge-equivariant GNN head).

Strategy: edges partitioned across 8 cores by dst range (graph partitioning),
so each core's segment-sum is purely local (no collectives).

- Math reformulation: each per-edge message is a linear combination of 9
  per-src-node scalars (a 48->9 projection of x) with cos/sin coefficients of
  (t, g-t, 2t-g), derived on device from sin ACT lookups via trig identities.
- Node projection table: host ships x pre-transposed ([48, VPAD]); the device
  runs one small matmul per 128-node tile (no PE transposes) and stores the
  table as 16-f32 node slots, 4 nodes per 256B row (row = n//4, slot = n%4),
  so each 32-tile chunk is ONE contiguous DMA store.
- Per-edge pipeline: dma_gather of src rows round-robin over 4 SWDGE queues
  (the Q7 descriptor-generation runs on a different core pair per queue, so
  gathers overlap ~4x) -> 4-way slot extraction -> trig + linear combination
  (DVE/ACT) into a persistent message stream.
- Segment sum without scatter: host sorts each core's edges by local dst and
  lays tokens out so scan order j maps to grid (p=j//L, c=j%L). Per-partition
  prefix scan + cross-partition offset fixup gives the global cumsum C;
  per-node sums are C[end_v] - C[start_v], fetched with a small dma_gather
  over the C stream + 16-way sub-row extraction.
- Self terms: direct per-tile matmuls of the own-node x slice (host-aligned)
  against the self-kernel weights; no gather.
- Finalize: mean-normalize, add self terms, sigmoid gate, project on (e1,e2).
"""
import sys

sys.path.insert(0, "/opt/trn_rl_repo")

import numpy as np

import concourse.bass as bass
import concourse.mybir as mybir
import concourse.tile as tile
import concourse.bacc as bacc
from concourse import bass_utils

F32 = mybir.dt.float32
I16 = mybir.dt.int16
AF = mybir.ActivationFunctionType
OP = mybir.AluOpType


def _ru(x, m):
    return (x + m - 1) // m * m


class Cfg:
    def __init__(self, V, E, n_cores=8):
        assert V % (n_cores * 4) == 0
        self.V, self.E, self.NCORES = V, E, n_cores
        self.VPAD = _ru(V, 512)
        self.NT = self.VPAD // 128          # node tiles
        self.TROWS = self.VPAD // 4         # table rows (4 nodes/row, 256B)
        assert self.TROWS <= 32768
        self.GE = 64                        # table row width (f32) = 256B
        self.OWN = V // n_cores
        self.OWNPAD = _ru(self.OWN + 1, 128)
        self.TOWN = self.OWNPAD // 128
        self.GB = 8192                      # gather batch tokens
        worst = E // n_cores + 8 * int(np.sqrt(E / n_cores)) + 256
        self.E_PAD = _ru(worst, self.GB)
        self.NBATCH = self.E_PAD // self.GB
        self.L = self.E_PAD // 128          # scan columns per partition
        self.NB_B = 2 * self.OWNPAD         # boundary tokens (B0, B1)
        assert self.E_PAD // 16 <= 32768
        self.CROWS = self.E_PAD // 16       # C table rows (16 tokens/row)
        self.CH = 8                         # phase-A tiles per chunk


FULL = Cfg(100000, 1600000)

_NC_CACHE = {}


def build_nc(cfg):
    key = (cfg.V, cfg.E)
    if key in _NC_CACHE:
        return _NC_CACHE[key]
    nc = bacc.Bacc("TRN2", target_bir_lowering=False, debug=False,
                   num_devices=cfg.NCORES, num_swdge_queues=4)

    xT = nc.dram_tensor("xT", [48, cfg.VPAD], F32, kind="ExternalInput")
    xTo = nc.dram_tensor("xTo", [48, cfg.OWNPAD], F32, kind="ExternalInput")
    w48 = nc.dram_tensor("w48", [48, 16], F32, kind="ExternalInput")
    wself = nc.dram_tensor("wself", [48, 4], F32, kind="ExternalInput")
    gidx = nc.dram_tensor("gidx", [128, cfg.E_PAD // 16], I16, kind="ExternalInput")
    sel8 = nc.dram_tensor("sel8", [128, cfg.E_PAD // 128], F32, kind="ExternalInput")
    antr = nc.dram_tensor("antr", [128, 2 * (cfg.E_PAD // 128)], F32, kind="ExternalInput")
    bidx = nc.dram_tensor("bidx", [128, cfg.NB_B // 16], I16, kind="ExternalInput")
    bsub = nc.dram_tensor("bsub", [128, cfg.NB_B // 128], F32, kind="ExternalInput")
    e1b = nc.dram_tensor("e1b", [128, cfg.TOWN * 3], F32, kind="ExternalInput")
    e2b = nc.dram_tensor("e2b", [128, cfg.TOWN * 3], F32, kind="ExternalInput")

    out = nc.dram_tensor("out", [128, cfg.TOWN * 3], F32, kind="ExternalOutput")

    GE = cfg.GE
    CH = cfg.CH
    with tile.TileContext(nc) as tc:
        with (
            tc.tile_pool(name="const", bufs=1) as cp,
            tc.tile_pool(name="dram", bufs=1, space="DRAM") as dp,
            tc.tile_pool(name="psF", bufs=1, space="PSUM") as psF,
            tc.tile_pool(name="gth", bufs=4) as gp,
            tc.tile_pool(name="edg", bufs=4) as edp,
            tc.tile_pool(name="act2", bufs=2) as a2p,
            tc.tile_pool(name="trg", bufs=1) as trp,
            tc.tile_pool(name="stream", bufs=1) as smp,
            tc.tile_pool(name="fin", bufs=1) as fp,
        ):
            w48_t = cp.tile([48, 16], F32)
            nc.sync.dma_start(out=w48_t[:], in_=w48.ap())
            wself_t = cp.tile([48, 4], F32)
            nc.sync.dma_start(out=wself_t[:], in_=wself.ap())
            pi_t = cp.tile([128, 1], F32)
            nc.vector.memset(pi_t[:], np.pi)

            table = dp.tile([cfg.TROWS, GE], F32)
            ctab = dp.tile([cfg.CROWS, GE], F32)  # cumsum stream as 256B rows

            # ---------- Phase A: node projection table ----------
            # table row n//4, slot n%4: node slot = 16 f32 (9 used + pad).
            # Each CH-tile chunk stores as ONE contiguous DMA. Pools nested so
            # their SBUF frees before phase B.
            xo = fp.tile([128, cfg.TOWN * 4], F32)
            with (
                tc.tile_pool(name="xa", bufs=2) as xap,
                tc.tile_pool(name="stg", bufs=2) as stp,
                tc.tile_pool(name="psM", bufs=2, space="PSUM") as psM,
            ):
                for c0 in range(0, cfg.NT, CH):
                    nt = min(CH, cfg.NT - c0)
                    xc = xap.tile([48, CH * 128], F32, tag="xc")
                    nc.sync.dma_start(
                        out=xc[:, : nt * 128],
                        in_=xT.ap()[:, c0 * 128:(c0 + nt) * 128],
                    )
                    pM = psM.tile([128, CH * 16], F32, tag="pM")
                    for i in range(nt):
                        nc.tensor.matmul(
                            out=pM[:, i * 16:(i + 1) * 16],
                            lhsT=xc[:, i * 128:(i + 1) * 128],
                            rhs=w48_t[:],
                            start=True, stop=True,
                        )
                    stg = stp.tile([128, CH * 16], F32, tag="stg")
                    nc.vector.tensor_copy(out=stg[:, : nt * 16], in_=pM[:, : nt * 16])
                    dst = bass.AP(table[:].tensor, c0 * 128 * 16,
                                  [[16, 128], [128 * 16, nt], [1, 16]])
                    src = stg[:, : nt * 16].rearrange("p (t u) -> p t u", u=16)
                    nc.sync.dma_start(out=dst, in_=src)

                # ---- own-node self terms (direct matmuls, no gather) ----
                for s0 in range(0, cfg.TOWN, CH):
                    st_ = min(CH, cfg.TOWN - s0)
                    xoc = xap.tile([48, CH * 128], F32, tag="xc")
                    nc.sync.dma_start(
                        out=xoc[:, : st_ * 128],
                        in_=xTo.ap()[:, s0 * 128:(s0 + st_) * 128],
                    )
                    pS = psM.tile([128, CH * 16], F32, tag="pM")
                    for i in range(st_):
                        nc.tensor.matmul(
                            out=pS[:, i * 4:(i + 1) * 4],
                            lhsT=xoc[:, i * 128:(i + 1) * 128],
                            rhs=wself_t[:],
                            start=True, stop=True,
                        )
                    nc.vector.tensor_copy(out=xo[:, s0 * 4:(s0 + st_) * 4],
                                          in_=pS[:, : st_ * 4])
            xo3 = xo[:].rearrange("p (b u) -> p b u", u=4)

            # preload the whole gather-index stream once (no per-batch waits)
            giall = smp.tile([128, cfg.E_PAD // 16], I16)
            nc.sync.dma_start(out=giall[:], in_=gidx.ap())

            # persistent message stream [128, L, 4]
            msg = smp.tile([128, cfg.L * 4], F32)
            m4 = msg[:].rearrange("p (c e) -> p c e", e=4)

            # ---------- Phase B: edge tiles (2 gather batches per tile) ----
            # Gather + slot-extraction run per 8192-token batch (4 SWDGE
            # queues round-robin, small gt buffers); trig + message math run
            # at 2-batch tile width to halve DVE op overhead.
            NBL = cfg.GB // 128          # token columns per gather batch (64)
            NTILE = (cfg.NBATCH + 1) // 2
            for t in range(NTILE):
                nb = min(2, cfg.NBATCH - 2 * t)
                W = nb * NBL             # token columns this tile
                tok0 = 2 * t * NBL
                sel = edp.tile([128, 2 * NBL], F32, tag="sel")
                nc.sync.dma_start(out=sel[:, :W], in_=sel8.ap()[:, tok0:tok0 + W])
                atr = edp.tile([128, 4 * NBL], F32, tag="atr")
                nc.sync.dma_start(out=atr[:, : 2 * W],
                                  in_=antr.ap()[:, 2 * tok0:2 * tok0 + 2 * W])

                # replicated-sel mask source, tile width
                msf = trp.tile([128, 2 * NBL * 9], F32, tag="msf")
                ms3 = msf[:, : W * 9].rearrange("p (b u) -> p b u", u=9)
                nc.vector.tensor_copy(out=ms3[:, :, 0:1],
                                      in_=sel[:, :W].unsqueeze(2))
                for (src_w, dst0, w) in ((1, 1, 1), (2, 2, 2), (4, 4, 4), (1, 8, 1)):
                    nc.vector.tensor_copy(out=ms3[:, :, dst0:dst0 + w],
                                          in_=ms3[:, :, 0:src_w])

                ext = trp.tile([128, 2 * NBL * 9], F32, tag="ext")
                for h in range(nb):
                    b = 2 * t + h
                    gt = gp.tile([128, NBL * GE], F32, tag="gt")
                    nc.gpsimd.dma_gather(
                        out_ap=gt[:].rearrange("p (b e) -> p b e", e=GE),
                        in_ap=table[:],
                        idxs_ap=giall[:, b * (cfg.GB // 16):(b + 1) * (cfg.GB // 16)],
                        num_idxs=cfg.GB, num_idxs_reg=cfg.GB, elem_size=GE,
                        single_packet=False, queue_num=b % 4,
                    )
                    g3 = gt[:].rearrange("p (b e) -> p b e", e=GE)
                    # 4-way slot extraction into ext[:, h-half]
                    sl = slice(h * NBL * 9, (h + 1) * NBL * 9)
                    msf_h = msf[:, sl]
                    ext_h = ext[:, sl]
                    e3h = ext_h.rearrange("p (b u) -> p b u", u=9)
                    mk = trp.tile([128, NBL * 9], F32, tag="mkf")
                    mk3 = mk[:].rearrange("p (b u) -> p b u", u=9)
                    nc.vector.tensor_scalar(out=mk[:], in0=msf_h, scalar1=0.0,
                                            scalar2=None, op0=OP.is_equal)
                    nc.vector.tensor_tensor(out=e3h, in0=g3[:, :, 0:9], in1=mk3,
                                            op=OP.mult)
                    for k in (1, 2, 3):
                        nc.vector.tensor_scalar(out=mk[:], in0=msf_h,
                                                scalar1=float(k), scalar2=None,
                                                op0=OP.is_equal)
                        nc.vector.tensor_tensor(out=mk3, in0=g3[:, :, 16 * k:16 * k + 9],
                                                in1=mk3, op=OP.mult)
                        nc.vector.tensor_tensor(out=ext_h, in0=ext_h,
                                                in1=mk[:], op=OP.add)
                e3 = ext[:, : W * 9].rearrange("p (b u) -> p b u", u=9)

                # trig: 2 wide ACT sins over [t | g], cos via half-angle
                sfull = a2p.tile([128, 4 * NBL], F32, tag="sfull")
                nc.scalar.activation(sfull[:, : 2 * W], atr[:, : 2 * W], AF.Sin,
                                     bias=pi_t[:], scale=-1.0)
                shalf = a2p.tile([128, 4 * NBL], F32, tag="shalf")
                nc.scalar.activation(shalf[:, : 2 * W], atr[:, : 2 * W], AF.Sin,
                                     bias=pi_t[:], scale=-0.5)
                cfull = trp.tile([128, 4 * NBL], F32, tag="cfull")
                nc.vector.tensor_tensor(out=cfull[:, : 2 * W],
                                        in0=shalf[:, : 2 * W],
                                        in1=shalf[:, : 2 * W], op=OP.mult)
                nc.vector.tensor_scalar(out=cfull[:, : 2 * W],
                                        in0=cfull[:, : 2 * W],
                                        scalar1=-2.0, scalar2=1.0,
                                        op0=OP.mult, op1=OP.add)
                st, sg = sfull[:, :W], sfull[:, W:2 * W]
                ct, cg = cfull[:, :W], cfull[:, W:2 * W]

                def tt_op(nm, a, bb, op):
                    o = trp.tile([128, 2 * NBL], F32, tag=nm)
                    nc.vector.tensor_tensor(out=o[:, :W], in0=a, in1=bb, op=op)
                    return o

                pA = tt_op("tU", cg, ct, OP.mult)
                pB = tt_op("tV", sg, st, OP.mult)
                cd = tt_op("cd", pA[:, :W], pB[:, :W], OP.add)
                pC = tt_op("tU", sg, ct, OP.mult)
                pD = tt_op("tV", cg, st, OP.mult)
                sd = tt_op("sd", pC[:, :W], pD[:, :W], OP.subtract)
                c2 = tt_op("c2", st, st, OP.mult)
                nc.vector.tensor_scalar(out=c2[:, :W], in0=c2[:, :W],
                                        scalar1=-2.0, scalar2=1.0,
                                        op0=OP.mult, op1=OP.add)
                s2 = trp.tile([128, 2 * NBL], F32, tag="s2")
                nc.vector.scalar_tensor_tensor(out=s2[:, :W], in0=st,
                                               scalar=2.0, in1=ct,
                                               op0=OP.mult, op1=OP.mult)
                qA = tt_op("tU", c2[:, :W], cg, OP.mult)
                qB = tt_op("tV", s2[:, :W], sg, OP.mult)
                chv = tt_op("chv", qA[:, :W], qB[:, :W], OP.add)
                qC = tt_op("tU", s2[:, :W], cg, OP.mult)
                qD = tt_op("tV", c2[:, :W], sg, OP.mult)
                shv = tt_op("shv", qC[:, :W], qD[:, :W], OP.subtract)

                def ch_(c):
                    return e3[:, :, c]

                m3 = m4[:, tok0:tok0 + W, :]
                tA = trp.tile([128, 2 * NBL], F32, tag="tA")
                tB = trp.tile([128, 2 * NBL], F32, tag="tB")

                def mul(o, a, bb):
                    nc.vector.tensor_tensor(out=o, in0=a, in1=bb, op=OP.mult)

                def add(o, a, bb):
                    nc.vector.tensor_tensor(out=o, in0=a, in1=bb, op=OP.add)

                def sub(o, a, bb):
                    nc.vector.tensor_tensor(out=o, in0=a, in1=bb, op=OP.subtract)

                tAW, tBW = tA[:, :W], tB[:, :W]
                # m0 = na + cd*zr - sd*zi
                mul(tAW, cd[:, :W], ch_(1))
                mul(tBW, sd[:, :W], ch_(2))
                sub(tAW, tAW, tBW)
                add(m3[:, :, 0], tAW, ch_(0))
                # mv1 = ct*sa - st*sb + cg*pr - sg*pi + ch*rr - sh*ri
                mul(tAW, ct, ch_(3))
                mul(tBW, st, ch_(4))
                sub(tAW, tAW, tBW)
                mul(tBW, cg, ch_(5))
                add(tAW, tAW, tBW)
                mul(tBW, sg, ch_(6))
                sub(tAW, tAW, tBW)
                mul(tBW, chv[:, :W], ch_(7))
                add(tAW, tAW, tBW)
                mul(tBW, shv[:, :W], ch_(8))
                sub(m3[:, :, 1], tAW, tBW)
                # mv2 = st*sa + ct*sb + sg*pr + cg*pi + sh*rr + ch*ri
                mul(tAW, st, ch_(3))
                mul(tBW, ct, ch_(4))
                add(tAW, tAW, tBW)
                mul(tBW, sg, ch_(5))
                add(tAW, tAW, tBW)
                mul(tBW, cg, ch_(6))
                add(tAW, tAW, tBW)
                mul(tBW, shv[:, :W], ch_(7))
                add(tAW, tAW, tBW)
                mul(tBW, chv[:, :W], ch_(8))
                add(m3[:, :, 2], tAW, tBW)
                # deg component = 1.0
                nc.vector.tensor_scalar(out=m3[:, :, 3], in0=ct,
                                        scalar1=0.0, scalar2=1.0,
                                        op0=OP.mult, op1=OP.add)

            # token at scan position 0 is the cumsum baseline: zero it
            nc.vector.memset(msg[0:1, 0:4], 0.0)

            # ---------- scan: per-partition inclusive cumsum + offsets ----
            for c in range(4):
                v = msg[:, c::4]
                nc.vector.tensor_tensor_scan(
                    out=v, data0=v, data1=v, initial=0.0,
                    op0=OP.add, op1=OP.bypass)
            tot = fp.tile([128, 4], F32)
            nc.vector.tensor_copy(out=tot[:], in_=msg[:, (cfg.L - 1) * 4:cfg.L * 4])
            ident = cp.tile([128, 128], F32)
            from concourse.masks import make_identity
            make_identity(nc, ident[:])
            pTot = psF.tile([4, 128], F32, tag="pTot")
            nc.tensor.transpose(out=pTot[:], in_=tot[:], identity=ident[:])
            totT = fp.tile([4, 128], F32)
            nc.vector.tensor_copy(out=totT[:], in_=pTot[:])
            scT = fp.tile([4, 128], F32)
            nc.vector.tensor_tensor_scan(
                out=scT[:], data0=totT[:], data1=totT[:], initial=0.0,
                op0=OP.add, op1=OP.bypass)
            nc.vector.tensor_tensor(out=scT[:], in0=scT[:], in1=totT[:],
                                    op=OP.subtract)  # exclusive
            pOff = psF.tile([128, 4], F32, tag="pOff")
            nc.tensor.transpose(out=pOff[:], in_=scT[:], identity=ident[0:4, 0:4])
            off = fp.tile([128, 4], F32)
            nc.vector.tensor_copy(out=off[:], in_=pOff[:])
            for c in range(4):
                nc.vector.tensor_scalar(
                    out=msg[:, c::4], in0=msg[:, c::4],
                    scalar1=off[:, c:c + 1], scalar2=None, op0=OP.add)

            # store C stream to DRAM: token j = p*L + c at flat j*4
            cflat = bass.AP(ctab[:].tensor, 0,
                            [[cfg.L * 4, 128], [1, cfg.L * 4]])
            nc.sync.dma_start(out=cflat, in_=msg[:])

            # ---------- boundary gather: B0/B1 per own node ----------
            gib = fp.tile([128, cfg.NB_B // 16], I16)
            nc.sync.dma_start(out=gib[:], in_=bidx.ap())
            bsu0 = fp.tile([128, cfg.NB_B // 128], F32)
            nc.sync.dma_start(out=bsu0[:], in_=bsub.ap())
            bsu = fp.tile([128, (cfg.NB_B // 128) * 4], F32)
            bs3 = bsu[:].rearrange("p (b e) -> p b e", e=4)
            nc.vector.tensor_copy(out=bs3[:, :, 0:1],
                                  in_=bsu0[:].unsqueeze(2))
            nc.vector.tensor_copy(out=bs3[:, :, 1:2], in_=bs3[:, :, 0:1])
            nc.vector.tensor_copy(out=bs3[:, :, 2:4], in_=bs3[:, :, 0:2])
            NBC = cfg.NB_B // 128            # boundary token columns (2*TOWN)
            bval = fp.tile([128, NBC * 4], F32)
            bv3 = bval[:].rearrange("p (b e) -> p b e", e=4)
            pos = 0
            q = 0
            while pos < cfg.NB_B:
                n = min(cfg.GB, cfg.NB_B - pos)
                ncol = n // 128
                c0 = pos // 128
                gt = gp.tile([128, (cfg.GB // 128) * GE], F32, tag="gt")
                nc.gpsimd.dma_gather(
                    out_ap=gt[:, : ncol * GE].rearrange("p (b e) -> p b e", e=GE),
                    in_ap=ctab[:],
                    idxs_ap=gib[:, pos // 16:(pos + n) // 16],
                    num_idxs=n, num_idxs_reg=n, elem_size=GE,
                    single_packet=False, queue_num=q % 4,
                )
                q += 1
                gb4 = gt[:, : ncol * GE].rearrange(
                    "p (b s e) -> p b s e", s=16, e=4)
                bm = fp.tile([128, (cfg.GB // 128) * 4], F32, tag="bbm")
                bm3 = bm[:, : ncol * 4].rearrange("p (b e) -> p b e", e=4)
                bt = fp.tile([128, (cfg.GB // 128) * 4], F32, tag="bbt")
                bt3 = bt[:, : ncol * 4].rearrange("p (b e) -> p b e", e=4)
                bsl = bsu[:, c0 * 4:(c0 + ncol) * 4]
                for s in range(16):
                    nc.vector.tensor_scalar(
                        out=bm[:, : ncol * 4], in0=bsl,
                        scalar1=float(s), scalar2=None, op0=OP.is_equal)
                    if s == 0:
                        nc.vector.tensor_tensor(
                            out=bv3[:, c0:c0 + ncol, :], in0=gb4[:, :, 0, :],
                            in1=bm3, op=OP.mult)
                    else:
                        nc.vector.tensor_tensor(out=bt3, in0=gb4[:, :, s, :],
                                                in1=bm3, op=OP.mult)
                        nc.vector.tensor_tensor(
                            out=bv3[:, c0:c0 + ncol, :],
                            in0=bv3[:, c0:c0 + ncol, :], in1=bt3, op=OP.add)
                pos += n
            # per-node sums: B1 - B0  ([128, TOWN, 4])
            b0 = bv3[:, 0:cfg.TOWN, :]
            b1 = bv3[:, cfg.TOWN:2 * cfg.TOWN, :]
            acc = fp.tile([128, cfg.TOWN * 4], F32)
            a3 = acc[:].rearrange("p (b e) -> p b e", e=4)
            nc.vector.tensor_tensor(out=a3, in0=b1, in1=b0, op=OP.subtract)

            # ---------- finalize ----------
            deg = fp.tile([128, cfg.TOWN], F32)
            nc.vector.tensor_scalar(out=deg[:], in0=a3[:, :, 3], scalar1=1.0,
                                    scalar2=None, op0=OP.max)
            inv = fp.tile([128, cfg.TOWN], F32)
            nc.vector.reciprocal(out=inv[:], in_=deg[:])

            e1t = fp.tile([128, cfg.TOWN * 3], F32)
            nc.sync.dma_start(out=e1t[:], in_=e1b.ap())
            e2t = fp.tile([128, cfg.TOWN * 3], F32)
            nc.sync.dma_start(out=e2t[:], in_=e2b.ap())

            mag = fp.tile([128, cfg.TOWN], F32)
            nc.vector.tensor_tensor(out=mag[:], in0=a3[:, :, 0], in1=inv[:], op=OP.mult)
            nc.vector.tensor_tensor(out=mag[:], in0=mag[:], in1=xo3[:, :, 0], op=OP.add)
            t1 = fp.tile([128, cfg.TOWN], F32)
            nc.vector.tensor_tensor(out=t1[:], in0=a3[:, :, 1], in1=inv[:], op=OP.mult)
            nc.vector.tensor_tensor(out=t1[:], in0=t1[:], in1=xo3[:, :, 1], op=OP.add)
            t2 = fp.tile([128, cfg.TOWN], F32)
            nc.vector.tensor_tensor(out=t2[:], in0=a3[:, :, 2], in1=inv[:], op=OP.mult)
            nc.vector.tensor_tensor(out=t2[:], in0=t2[:], in1=xo3[:, :, 2], op=OP.add)
            sgm = fp.tile([128, cfg.TOWN], F32)
            nc.scalar.activation(sgm[:], mag[:], AF.Sigmoid)

            ot = fp.tile([128, cfg.TOWN * 3], F32)
            o3 = ot[:].rearrange("p (b u) -> p b u", u=3)
            e13 = e1t[:].rearrange("p (b u) -> p b u", u=3)
            e23 = e2t[:].rearrange("p (b u) -> p b u", u=3)
            tX = fp.tile([128, cfg.TOWN], F32, tag="tX")
            for j in range(3):
                nc.vector.tensor_tensor(out=o3[:, :, j], in0=t1[:], in1=e13[:, :, j], op=OP.mult)
                nc.vector.tensor_tensor(out=tX[:], in0=t2[:], in1=e23[:, :, j], op=OP.mult)
                nc.vector.tensor_tensor(out=o3[:, :, j], in0=o3[:, :, j], in1=tX[:], op=OP.add)
                nc.vector.tensor_tensor(out=o3[:, :, j], in0=o3[:, :, j], in1=sgm[:], op=OP.mult)
            nc.sync.dma_start(out=out.ap(), in_=ot[:])

    nc.finalize()
    _NC_CACHE[key] = nc
    return nc


def _wrap16(tok, epad):
    a = np.zeros(epad, dtype=np.int16)
    a[: len(tok)] = tok
    a = a.reshape(epad // 16, 16).T.copy()       # token i -> [i%16, i//16]
    return np.tile(a, (8, 1))


def _toklay(v, epad, fill=0.0, dtype=np.float32):
    a = np.full(epad, fill, dtype=dtype)
    a[: len(v)] = v
    return a.reshape(epad // 128, 128).T.copy()  # token i -> [i%128, i//128]


def pack_inputs(cfg, x, edge_index, angles, transporters, e1, e2,
                w_self0, w_n00, w_n10, w_self11, w_n01, w_n11):
    V = cfg.V
    C0 = C1 = 16
    # neighbor-kernel projection: 9 channels (na zr zi sa sb pr pi rr ri)
    W = np.zeros((48, 16), dtype=np.float32)
    w10a, w10b = w_n10[:, 0], w_n10[:, 1]
    p_, q_, r_, s_ = w_n11[:, 0], w_n11[:, 1], w_n11[:, 2], w_n11[:, 3]
    k = np.arange(C1)
    a1i, a2i = 16 + 2 * k, 17 + 2 * k
    W[a1i, 1] = w10a; W[a2i, 1] = w10b
    W[a2i, 2] = w10a; W[a1i, 2] = -w10b
    W[a1i, 5] = p_;   W[a2i, 5] = -q_
    W[a2i, 6] = p_;   W[a1i, 6] = q_
    W[a1i, 7] = r_;   W[a2i, 7] = s_
    W[a1i, 8] = s_;   W[a2i, 8] = -r_
    W[:C0, 0] = w_n00
    W[:C0, 3] = w_n01[:, 0]
    W[:C0, 4] = w_n01[:, 1]
    # self-kernel projection: (mag, t1, t2, 0)
    Ws = np.zeros((48, 4), dtype=np.float32)
    sa_, sb_ = w_self11[:, 0], w_self11[:, 1]
    Ws[:C0, 0] = w_self0
    Ws[a1i, 1] = sa_; Ws[a2i, 1] = -sb_
    Ws[a2i, 2] = sa_; Ws[a1i, 2] = sb_

    xpad = np.zeros((cfg.VPAD, 48), dtype=np.float32)
    xpad[:V] = x
    xTn = xpad.T.copy()                           # [48, VPAD]

    src = np.asarray(edge_index[0]).astype(np.int64)
    dst = np.asarray(edge_index[1]).astype(np.int64)
    ang = np.asarray(angles).astype(np.float32)
    trf = np.asarray(transporters).astype(np.float32)

    # token i (gather layout) <-> scan position j: j = (i%128)*L + i//128
    epad = cfg.E_PAD
    L = cfg.L
    i_all = np.arange(epad)
    j_of_i = (i_all % 128) * L + i_all // 128

    in_maps = []
    for c in range(cfg.NCORES):
        lo, hi = c * cfg.OWN, (c + 1) * cfg.OWN
        ids = np.nonzero((dst >= lo) & (dst < hi))[0]
        dl = (dst[ids] - lo).astype(np.int64)
        order = np.argsort(dl, kind="stable")
        eidx = ids[order]
        dls = dl[order]
        n = len(eidx)
        if n + 1 > epad:
            raise RuntimeError("edge shard exceeds E_PAD")
        gj = np.zeros(epad, dtype=np.int16)
        sj = np.zeros(epad, dtype=np.int8)
        aj = np.zeros(epad, dtype=np.float32)
        tj = np.zeros(epad, dtype=np.float32)
        gj[1:n + 1] = (src[eidx] // 4).astype(np.int16)
        sj[1:n + 1] = (src[eidx] % 4).astype(np.int8)
        aj[1:n + 1] = ang[eidx]
        tj[1:n + 1] = trf[eidx]
        g_tok = gj[j_of_i]
        s_tok = sj[j_of_i]
        a_tok = aj[j_of_i]
        t_tok = tj[j_of_i]

        # boundaries: inclusive-cumsum positions per node (scan positions)
        rowptr = np.searchsorted(dls, np.arange(cfg.OWN + 1))
        b0 = np.zeros(cfg.OWNPAD, dtype=np.int64)
        b1 = np.zeros(cfg.OWNPAD, dtype=np.int64)
        b0[: cfg.OWN] = rowptr[:-1]
        b1[: cfg.OWN] = rowptr[1:]
        btok = np.concatenate([b0, b1])
        bidx_np = _wrap16((btok // 16).astype(np.int16), cfg.NB_B)
        bsub_np = _toklay((btok % 16).astype(np.float32), cfg.NB_B)

        xop = np.zeros((cfg.OWNPAD, 48), dtype=np.float32)
        xop[: cfg.OWN] = x[lo:hi]
        xToc = xop.T.copy()                       # [48, OWNPAD]

        def blk(a):
            return a.reshape(cfg.TOWN, 128, 3).transpose(1, 0, 2).reshape(128, -1).copy()

        e1p = np.zeros((cfg.OWNPAD, 3), dtype=np.float32)
        e1p[: cfg.OWN] = 2.0 * np.asarray(e1[lo:hi], dtype=np.float32)
        e2p = np.zeros((cfg.OWNPAD, 3), dtype=np.float32)
        e2p[: cfg.OWN] = 2.0 * np.asarray(e2[lo:hi], dtype=np.float32)

        A = _toklay(a_tok, epad)
        T = _toklay(t_tok, epad)
        ncols = epad // 128
        antr_np = np.zeros((128, 2 * ncols), dtype=np.float32)
        NBLh = cfg.GB // 128
        tpos = 0
        while tpos < ncols:
            Wc = min(2 * NBLh, ncols - tpos)
            antr_np[:, 2 * tpos:2 * tpos + Wc] = A[:, tpos:tpos + Wc]
            antr_np[:, 2 * tpos + Wc:2 * tpos + 2 * Wc] = T[:, tpos:tpos + Wc]
            tpos += Wc
        in_maps.append({
            "xT": xTn, "xTo": xToc, "w48": W, "wself": Ws,
            "gidx": _wrap16(g_tok, epad),
            "sel8": _toklay(s_tok.astype(np.float32), epad),
            "antr": antr_np,
            "bidx": bidx_np, "bsub": bsub_np,
            "e1b": blk(e1p), "e2b": blk(e2p),
        })
    return in_maps


def unshard(cfg, results):
    out = np.zeros((cfg.V, 3), dtype=np.float32)
    for c, res in enumerate(results):
        o = res["out"].reshape(128, cfg.TOWN, 3).transpose(1, 0, 2).reshape(-1, 3)
        out[c * cfg.OWN:(c + 1) * cfg.OWN] = o[: cfg.OWN]
    return out


def kernel(**inputs):
    cfg = FULL
    nc = build_nc(cfg)
    in_maps = pack_inputs(cfg, **inputs)
    res = bass_utils.run_bass_kernel_spmd(
        nc, in_maps, core_ids=list(range(cfg.NCORES)))
    return unshard(cfg, [r for r in res.results])
